# revision 58
# baseline (speedup 1.0000x reference)
"""Multi-head attention (B=2, S=2048, D=2048, H=16, causal+RoPE) on 8 trn2
NeuronCores, tensor-parallel over heads (2 heads per core), NO collectives.

Pipeline per core (heads 2c, 2c+1):
  P1: qkv projection in fp32r. Q^T/K^T feature-major [dh, t]; V natural
      [t, dh] cast to bf16 at the PSUM drain (DVE). RoPE on-chip:
      rotate-half via partition-strided SBUF-SBUF DMA, elementwise combine
      on gpsimd; one shared (unscaled) rope table, since
      sqrt(dh)*attn_scale[h] is folded into wq on the host (rope is
      linear, so the scale commutes into the scores). Activations stream
      on the ACT DMA queue, parallel to the weights on the sync queue.
  P2: attention per (head, batch), per 128-row q block, single score pass:
      scores [q,k] (fp32r) into PSUM chunks, diagonal chunk FIRST so its
      in-place mask + reduce overlap the remaining score matmuls;
      P = Exp on ACT with bias = -rowmax and Z accumulated via accum_out;
      P^T by PE transpose (identity operand); PV accumulates unnormalized
      A^T [dh, q]; the at-copy multiplies by the broadcast 1/Z built
      off-path on gpsimd. PV + at-copy of block qi are emitted in block
      qi+1's slot so the PE backfills the reduce/exp latency.
  P3: partial out_proj against this core's 256-column slice of w_out^T.
      Batch 0's blocks run "thin" (single PSUM bank), interleaved into
      batch 1's attention slots as additional PE backfill; batch 1's
      blocks run at full width at the end. Each core emits a full-shape
      [T, D] bf16 partial; the host sums the 8 partials in f32 (the
      all-reduce of the TP decomposition). No cross-core dependency
      exists anywhere in the NEFF, so per-core execution span is immune
      to dispatch skew.

All phases share one PSUM pool with eight [128,512] bank tags (g0..g7),
so phase transitions hand banks over per-tag instead of via pool
barriers.

Precision: q/k path fp32r, v/p/out_proj path bf16, 1/Z broadcast in f32
-> ~0.4-0.5% rel err (Frobenius) vs the fp32 reference.
"""
import math
import os

import numpy as np
import ml_dtypes

import concourse.bass as bass
import concourse.mybir as mybir
import concourse.tile as tile
from concourse import bacc
from concourse.bass_utils import run_bass_kernel_spmd

F32 = mybir.dt.float32
F32R = mybir.dt.float32r
BF16 = mybir.dt.bfloat16
AX = mybir.AxisListType.X
EXP = mybir.ActivationFunctionType.Exp
CPY = mybir.ActivationFunctionType.Copy

B, S, D = 2, 2048, 2048
H, DH = 16, 128
NC = 8
T = B * S              # 4096 flat tokens
NT = T // 512          # 8 token tiles of 512
ND = D // 128          # 16 contraction tiles
NQT = S // 128         # 16 q-tiles per batch
TOK = T // NC          # 512 tokens per core

LAST_RESULT = None     # BassKernelResults of the most recent run (for tests)

# experiment knobs (sim sweeps); defaults are the shipping configuration
KNOB_BACKFILL = int(os.environ.get("K_BACKFILL", "1"))   # thin P3 into P2
KNOB_AP_ALT = int(os.environ.get("K_AP_ALT", "0"))       # PV bank alternates
KNOB_DIAG_FIRST = int(os.environ.get("K_DIAG_FIRST", "1"))
KNOB_DEFER_PV = int(os.environ.get("K_DEFER_PV", "1"))   # PV in next slot


def _round_f32r(a):
    """fp32r rounds matmul inputs to 11 explicit mantissa bits; pre-round on
    host so the device DMA can feed f32r tiles without a cast pass."""
    u = np.ascontiguousarray(a, np.float32).view(np.uint32)
    u = ((u + np.uint32(1 << 11)) >> 12) << 12
    return u.view(np.float32)


def _bank(ps, i):
    """One PSUM bank by global tag; all phases share these eight tags."""
    return ps.tile([128, 512], F32, tag=f"g{i}", bufs=1, name=f"g{i}")


def _build(r1=1, r2=1, r3=1):
    """Build the SPMD program. r1 repeats phase 1, r2 the merged
    attention+out_proj section (phase-attribution benchmarking; 1=normal).
    r3 is accepted for interface compatibility and folded into r2."""
    r2 = max(r2, r3)
    nc = bacc.Bacc("TRN2", target_bir_lowering=False, debug=False,
                   num_devices=NC)

    xt_d = nc.declare_dram_parameter("xt", [D, T], F32R, isOutput=False)
    wqk_d = nc.declare_dram_parameter("wqk", [D, 512], F32R, isOutput=False)
    wv_d = nc.declare_dram_parameter("wv", [D, 256], F32R, isOutput=False)
    tabs_d = nc.declare_dram_parameter("tabs", [2, 128, 4, 512], F32,
                                       isOutput=False)
    masks_d = nc.declare_dram_parameter("cmask", [4, 128, 512], F32,
                                        isOutput=False)
    wout_d = nc.declare_dram_parameter("wout", [256, D], BF16, isOutput=False)
    identb_d = nc.declare_dram_parameter("identb", [128, 128], BF16,
                                         isOutput=False)
    identr_d = nc.declare_dram_parameter("identr", [128, 128], F32R,
                                         isOutput=False)
    o_d = nc.declare_dram_parameter("o", [T, D], BF16, isOutput=True)
    # DRAM bounce buffers for the rope rotate-half gather: the strided
    # cross-partition read is a plain (tracked) row-strided AP on the DRAM
    # side, unlike the SBUF-SBUF form which the dependency tracker cannot
    # see (and which the scheduler is then free to mis-order).
    rawd = [nc.dram_tensor(f"rawd{i}", [128, 4, 512], F32) for i in range(2)]

    with tile.TileContext(nc) as tc:
        with tc.tile_pool(name="res", bufs=1) as res, \
             tc.tile_pool(name="ps", bufs=1, space="PSUM") as ps:
            # resident across phases
            v_sb = res.tile([128, 32 * 256], BF16)        # [t%128, ttile*256+f]
            at = [[res.tile([128, S], BF16, name=f"at{h}b{b}", tag=f"at{h}{b}")
                   for b in range(B)] for h in range(2)]
            identb = res.tile([128, 128], BF16)
            identr = res.tile([128, 128], F32R)
            mask_sb = res.tile([128, 4, 512], F32)

            with tc.tile_pool(name="qkt", bufs=1) as qkt:
                qt = [qkt.tile([128, T], F32R, name=f"qt{h}", tag=f"qt{h}")
                      for h in range(2)]
                kt = [qkt.tile([128, T], F32R, name=f"kt{h}", tag=f"kt{h}")
                      for h in range(2)]
                qkres = qt + kt

                # ---------------- P1: projection + rope ----------------
                with tc.tile_pool(name="p1", bufs=1) as p1:
                    wqk_sb = p1.tile([128, ND, 512], F32R)
                    wv_sb = p1.tile([128, ND, 256], F32R)
                    for g in range(4):   # interleave so dd=0 chunks land first
                        nc.sync.dma_start(
                            wqk_sb[:, 4 * g:4 * g + 4, :],
                            wqk_d[512 * g:512 * (g + 1), :].rearrange(
                                "(a p) f -> p a f", p=128))
                        nc.sync.dma_start(
                            wv_sb[:, 4 * g:4 * g + 4, :],
                            wv_d[512 * g:512 * (g + 1), :].rearrange(
                                "(a p) f -> p a f", p=128))
                    tab_sb = p1.tile([128, 2, 4, 512], F32)
                    nc.sync.dma_start(tab_sb[:],
                                      tabs_d.rearrange("c p s f -> p c s f"))

                    for _ in range(r1):
                        for tt in range(NT):
                            soff = tt % 4       # position block in batch

                            psq = [_bank(ps, f) for f in range(4)]
                            psv = [_bank(ps, 4 + i) for i in range(4)]
                            for g in range(4):      # 4 d-tiles per DMA
                                xt = p1.tile([128, 4, 512], F32R, tag="xt",
                                             bufs=2)
                                nc.scalar.dma_start(
                                    xt[:],
                                    xt_d[512 * g:512 * (g + 1),
                                         tt * 512:(tt + 1) * 512]
                                    .rearrange("(a p) t -> p a t", p=128))
                                for a in range(4):
                                    dd = 4 * g + a
                                    for f in range(4):
                                        nc.tensor.matmul(
                                            psq[f][:],
                                            wqk_sb[:, dd,
                                                   f * 128:(f + 1) * 128],
                                            xt[:, a, :], start=(dd == 0),
                                            stop=(dd == ND - 1))
                                    for s_ in range(4):
                                        nc.tensor.matmul(
                                            psv[s_][:, :256],
                                            xt[:, a, s_ * 128:(s_ + 1) * 128],
                                            wv_sb[:, dd, :],
                                            start=(dd == 0),
                                            stop=(dd == ND - 1))

                            # V drains on DVE
                            for s_ in range(4):
                                gti = tt * 4 + s_   # global 128-token tile
                                nc.vector.tensor_copy(
                                    v_sb[:, gti * 256:(gti + 1) * 256],
                                    psv[s_][:, :256])

                            # rope on q (f=0,1) and k (f=2,3): drain on DVE,
                            # rotate-half via a DRAM bounce (tracked APs),
                            # elementwise combine on gpsimd
                            raw4 = p1.tile([128, 4, 512], F32, tag="raw",
                                           bufs=1)
                            for f in range(4):
                                nc.vector.tensor_copy(raw4[:, f, :],
                                                      psq[f][:])
                            rd = rawd[tt % 2]
                            nc.sync.dma_start(rd[:], raw4[:])
                            rot4 = p1.tile([128, 4, 512], F32, tag="rot",
                                           bufs=1)
                            nc.sync.dma_start(rot4[0:64, :, :],
                                              rd[1:128:2, :, :])
                            nc.sync.dma_start(rot4[64:128, :, :],
                                              rd[0:128:2, :, :])
                            for f in range(4):
                                t1 = p1.tile([128, 512], F32, tag="t1", bufs=2)
                                nc.gpsimd.tensor_mul(t1[:], raw4[:, f, :],
                                                     tab_sb[:, 0, soff, :])
                                nc.gpsimd.tensor_mul(rot4[:, f, :],
                                                     rot4[:, f, :],
                                                     tab_sb[:, 1, soff, :])
                                nc.gpsimd.tensor_add(
                                    qkres[f][:, tt * 512:(tt + 1) * 512],
                                    t1[:], rot4[:, f, :])

                # constants for P2 (ACT queue; arrive well before first use)
                nc.scalar.dma_start(identb[:], identb_d[:])
                nc.scalar.dma_start(identr[:], identr_d[:])
                nc.scalar.dma_start(mask_sb[:],
                                    masks_d.rearrange("r p f -> p r f"))

                # -------- P2 + P3: attention, out_proj interleaved --------
                with tc.tile_pool(name="p23", bufs=1) as p23:
                    wout_sb = p23.tile([128, 2, D], BF16)
                    nc.sync.dma_start(
                        wout_sb[:], wout_d.rearrange("(a p) f -> p a f",
                                                     p=128))

                    def p3_block(b, st, thin):
                        r0 = (b * NQT + st) * 128
                        outt = p23.tile([128, D], BF16, tag="outt", bufs=2)
                        if thin:
                            for e in range(4):
                                op = _bank(ps, 7)
                                for hh in range(2):
                                    nc.tensor.matmul(
                                        op[:],
                                        at[hh][b][:, st * 128:(st + 1) * 128],
                                        wout_sb[:, hh,
                                                e * 512:(e + 1) * 512],
                                        start=(hh == 0), stop=(hh == 1))
                                nc.scalar.activation(
                                    outt[:, e * 512:(e + 1) * 512], op[:],
                                    CPY)
                        else:
                            ops = [_bank(ps, (st % 2) * 4 + e)
                                   for e in range(4)]
                            for hh in range(2):
                                for e in range(4):
                                    nc.tensor.matmul(
                                        ops[e][:],
                                        at[hh][b][:, st * 128:(st + 1) * 128],
                                        wout_sb[:, hh,
                                                e * 512:(e + 1) * 512],
                                        start=(hh == 0), stop=(hh == 1))
                            for e in range(4):
                                dst = outt[:, e * 512:(e + 1) * 512]
                                if e % 2 == 0:
                                    nc.vector.tensor_copy(dst, ops[e][:])
                                else:
                                    nc.scalar.activation(dst, ops[e][:], CPY)
                        nc.sync.dma_start(o_d[r0:r0 + 128, :], outt[:])

                    for _ in range(r2):
                        pending_p3 = []

                        def backfill():
                            if pending_p3:
                                b_, st_ = pending_p3.pop(0)
                                p3_block(b_, st_, thin=True)

                        for b in range(B):
                            for hh in range(2):
                                _attn(nc, p23, ps, qt[hh], kt[hh], v_sb,
                                      mask_sb, at[hh][b], hh, b, identb,
                                      identr,
                                      backfill if (b == 1 and KNOB_BACKFILL)
                                      else None)
                            if b == 0 and KNOB_BACKFILL:
                                pending_p3 = [(0, st) for st in range(NQT)]
                        # flush: anything not absorbed + all of batch 1
                        for b_, st_ in pending_p3:
                            p3_block(b_, st_, thin=True)
                        if not KNOB_BACKFILL:
                            for st in range(NQT):
                                p3_block(0, st, thin=False)
                        for st in range(NQT):
                            p3_block(1, st, thin=False)

    nc.finalize()
    return nc


def _attn(nc, p2, ps, qth, kth, v_sb, mask_sb, at_bh, hh, b, identb,
          identr, backfill):
    """Causal attention for one (head, batch): writes normalized A^T (bf16)
    into at_bh [128(dh), S]. Software-pipelined one block deep: block qi's
    PV + at-copy are emitted during block qi+1's reduce/exp latency; the
    optional backfill callback emits one thin out_proj block per odd slot
    as extra PE filler. sqrt(dh)*attn_scale is folded into wq on the host,
    so scores arrive pre-scaled."""
    boff = b * S
    pend = None

    def finish(p):
        qi_, nkt_, et_, ap__, rzb_ = p
        for kt in range(nkt_):
            gti = b * 16 + kt
            nc.tensor.matmul(
                ap__[:, :128],
                v_sb[:, gti * 256 + hh * 128:gti * 256 + (hh + 1) * 128],
                et_[:, kt * 128:(kt + 1) * 128],
                start=(kt == 0), stop=(kt == nkt_ - 1))
        nc.vector.tensor_mul(at_bh[:, qi_ * 128:(qi_ + 1) * 128],
                             ap__[:, :128], rzb_[:])

    for qi in range(NQT):               # 128-row q blocks
        nch = qi // 4 + 1               # 512-wide k chunks (causal)
        nkt = qi + 1                    # 128-wide k tiles
        # ---- single score pass: [q, k] chunks in PSUM, diagonal first ----
        cm = (p2.tile([128, 4], F32, tag="cm", bufs=2, name="cm")
              if nch > 1 else None)
        nm = p2.tile([128, 1], F32, tag="nm", bufs=2)
        scs = [None] * nch
        corder = ([nch - 1] + list(range(nch - 1)) if KNOB_DIAG_FIRST
                  else list(range(nch)))
        for c in corder:
            n = 512 if c < nch - 1 else 128 * (qi % 4 + 1)
            nw = max(n, 256)            # f32r matmul is 4x slower below 256
            sp = _bank(ps, c)
            nc.tensor.matmul(
                sp[:, :nw],
                qth[:, boff + qi * 128:boff + (qi + 1) * 128],
                kth[:, boff + c * 512:boff + c * 512 + nw],
                start=True, stop=True)
            if c == nch - 1:
                # only the 128-wide diagonal tile needs masking; the rest
                # of the chunk is fully visible
                nc.vector.tensor_add(sp[:, n - 128:n], sp[:, n - 128:n],
                                     mask_sb[:, qi % 4, n - 128:n])
            if nch == 1:                # single chunk: reduce straight to -max
                nc.vector.reduce_max(out=nm[:], in_=sp[:, :n], axis=AX,
                                     negate=True)
            else:
                nc.vector.reduce_max(out=cm[:, c:c + 1], in_=sp[:, :n],
                                     axis=AX)
            scs[c] = (sp, n)
        if nch > 1:
            nc.vector.reduce_max(out=nm[:], in_=cm[:, :nch], axis=AX,
                                 negate=True)

        # PE backfill: previous block's PV + at-copy, plus a thin P3 block
        if pend is not None:
            finish(pend)
        if backfill is not None and qi % 2 == 1:
            backfill()

        # ---- exp chunks (shifted, Z-accumulated) + transposes ----
        pq = p2.tile([128, 2048], BF16, tag="pq", bufs=2)
        zc = p2.tile([128, 4], F32, tag="zc", bufs=2)
        et = p2.tile([128, 2048], BF16, tag="et", bufs=2)
        for c, (sp, n) in enumerate(scs):
            nc.scalar.activation(pq[:, c * 512:c * 512 + n], sp[:, :n], EXP,
                                 bias=nm[:], accum_out=zc[:, c:c + 1])
            kts = list(range(4 * c, min(4 * c + 4, nkt)))
            w = 128 * len(kts)
            tp = _bank(ps, 4 + c % 2)
            for j, kt in enumerate(kts):
                nc.tensor.matmul(tp[:, j * 128:(j + 1) * 128],
                                 pq[:, kt * 128:(kt + 1) * 128], identb[:],
                                 start=True, stop=True)
            dst = et[:, 4 * c * 128:4 * c * 128 + w]
            if c % 2 == 0:
                nc.scalar.activation(dst, tp[:, :w], CPY)
            else:
                nc.vector.tensor_copy(dst, tp[:, :w])

        # off-path: Z -> 1/Z -> row (PE transpose through the ap bank's
        # corner) -> broadcast (consumed next slot)
        z = p2.tile([128, 1], F32, tag="z", bufs=2)
        nc.vector.reduce_sum(out=z[:], in_=zc[:, :nch], axis=AX)
        rz = p2.tile([128, 1], F32R, tag="rz", bufs=2)
        with nc.allow_low_precision(reason="1/Z read at 11-bit mantissa"):
            nc.vector.reciprocal(rz[:], z[:])
        ap_ = _bank(ps, 6 + qi % 2 if KNOB_AP_ALT else 6)
        nc.tensor.matmul(ap_[0:1, 128:256], rz[:], identr[:],
                         start=True, stop=True)
        rzr = p2.tile([1, 128], F32, tag="rzr", bufs=2)
        nc.scalar.activation(rzr[:], ap_[0:1, 128:256], CPY)
        rzb = p2.tile([128, 128], F32, tag="rzb", bufs=2)
        nc.gpsimd.partition_broadcast(rzb[:], rzr[0:1, :])
        pend = (qi, nkt, et, ap_, rzb)
        if not KNOB_DEFER_PV:
            finish(pend)
            pend = None
    if pend is not None:
        finish(pend)


_NC_CACHE = None


def prepare_in_maps(x, w_qkv, w_out, attn_scale):
    x = np.asarray(x, np.float32)
    w_qkv = np.asarray(w_qkv, np.float32)
    w_out = np.asarray(w_out, np.float32)
    attn_scale = np.asarray(attn_scale, np.float32)

    # host-side layout prep (sharding): feature-major activations
    xt = _round_f32r(x.reshape(T, D).T)                       # [D, T]
    # rope tables, feature-major, rotate-half sign folded into sin;
    # unscaled and shared by q and k
    inv = 1.0 / (10000.0 ** (np.arange(0, DH, 2, dtype=np.float32) / DH))
    th = np.outer(inv, np.arange(S, dtype=np.float32))        # [64, S]
    cosT = np.cos(np.concatenate([th, th], 0)).astype(np.float32)
    sinT = np.sin(np.concatenate([th, th], 0)).astype(np.float32)
    sinT[:64] *= -1.0
    tabs = np.stack([cosT.reshape(128, 4, 512),
                     sinT.reshape(128, 4, 512)])              # [2,128,4,512]
    # causal diag-block masks, [q, k] orientation
    kk = np.arange(512)[None, :]
    pp = np.arange(128)[:, None]
    masks = np.stack([np.where(kk <= 128 * r + pp, 0.0, -1e9)
                      for r in range(4)]).astype(np.float32)  # [4, 128, 512]
    woutT = np.ascontiguousarray(w_out.T).astype(ml_dtypes.bfloat16)

    in_maps = []
    for c in range(NC):
        h0 = 2 * c
        # sqrt(dh)*attn_scale folded into the q projection weights (rope is
        # linear, so the scale commutes through it into the scores)
        wq = w_qkv[128 * h0:128 * h0 + 256].copy()            # both heads' q
        wq[:128] *= math.sqrt(DH) * attn_scale[h0]
        wq[128:] *= math.sqrt(DH) * attn_scale[h0 + 1]
        wk = w_qkv[D + 128 * h0:D + 128 * h0 + 256]
        wv = w_qkv[2 * D + 128 * h0:2 * D + 128 * h0 + 256]
        wqk = _round_f32r(np.concatenate([wq, wk], 0).T)      # [D, 512]
        wvT = _round_f32r(wv.T)                               # [D, 256]
        in_maps.append({
            "xt": xt, "wqk": wqk, "wv": wvT, "tabs": tabs,
            "cmask": masks,
            "wout": np.ascontiguousarray(woutT[256 * c:256 * (c + 1), :]),
            "identb": np.eye(128, dtype=ml_dtypes.bfloat16),
            "identr": np.eye(128, dtype=np.float32),
        })
    return in_maps


def kernel(x, mask, w_qkv, w_out, attn_scale):
    global _NC_CACHE, LAST_RESULT
    in_maps = prepare_in_maps(x, w_qkv, w_out, attn_scale)
    if _NC_CACHE is None:
        _NC_CACHE = _build()
    res = run_bass_kernel_spmd(_NC_CACHE, in_maps, list(range(NC)))
    LAST_RESULT = res
    out = res.results[0]["o"].astype(np.float32)
    for c in range(1, NC):
        out += res.results[c]["o"].astype(np.float32)
    return out.reshape(B, S, D)
